# revision 31
# baseline (speedup 1.0000x reference)
"""Attention-pooling Trainium2 kernel (fp8 DoubleRow + split-engine exp).

Problem: out = mean_s(softmax((x@Wq+bq)(x@Wk+bk)^T / sqrt(E)) @ (x@Wv+bv))
with x [4, 4096, 256], output [4, 1, 256].

Math restructuring (exact up to fp reassociation):
  * mean_s(dist @ V) = (colsum(dist)/S) @ V  -- the second S x S matmul
    collapses to a length-S vector "w" and one matvec.
  * K bias drops (row-constant in scores); V bias folds to host "+bv".
  * Q/K projections fold into ONE projection: scores = x M x^T + u^T x^T
    with M = Wq Wk^T, u = Wk bq (host-computed E x E / E-sized weight prep).
    So the device never computes K.
  * Wv moves to the END: pooled = (w @ x) @ Wv -- the V projection
    (S x E x E) becomes an E x E matmul on a [1, E] vector.
  * Scores run in fp8(e4m3) with DoubleRow perf mode: the E=256
    contraction happens in ONE PE pass at 2x bf16 rate. M is pre-scaled
    by 16 host-side so fp8 operands sit in their sweet spot; the exp
    applies scale 1/256 and a constant -2 shift to keep exp outputs in
    range. Numerics validated in simulation: rel_err ~0.009 vs 2e-2 gate.
  * exp is split across engines: ACT computes true exp (with accum_out
    row-sums); DVE computes a Schraudolph-style exp -- one tensor_scalar
    (score*A + B) -> int16, whose bit pattern IS the bf16 exp
    approximation (+-3.5% sawtooth, washes out in the pooled mean).

Sharding: 8 cores = 4 batches x 2 query-row halves; x arrives rolled so
each core's 2048 query rows are columns 0:2047 (permutation-invariant
for the pooled result). Host sums the two halves per batch, /S, +bv.
"""

import numpy as np

import concourse.bass as bass  # noqa: F401
import concourse.mybir as mybir
import concourse.tile as tile
from concourse import bacc

B, S, E = 4, 4096, 256
HALF = S // 2          # query rows per core
P = 128
N_CORES = 8
QTILES = HALF // P     # 16
F32 = mybir.dt.float32
BF16 = mybir.dt.bfloat16
FP8 = mybir.dt.float8e4
I16 = mybir.dt.int16
DR = mybir.MatmulPerfMode.DoubleRow

CSHIFT = 2.0                       # exp(score - CSHIFT): keeps e4m3/bf16 in range
A_SCH = 128.0 / np.log(2.0)        # bf16 Schraudolph slope (per unit exp arg)
A2 = A_SCH / 256.0                 # folded score scale 1/256
B2 = (127 * 128 - 5.5) - CSHIFT * A_SCH
COLSUM_LAG = 2
# per-qtile chunking of the 4096 keys; chunk index -> (start, width).
# ACT chunks run true exp with accum_out rowsums; DVE chunks run the
# Schraudolph tensor_scalar with a bf16 reduce for their rowsum.
CHUNKS = [(0, 1536), (1536, 1536), (3072, 1024)]


def _dve_parts(qi):
    if qi in (14, 15):
        return ()          # tail qtiles all-ACT so recb isn't on the DVE backlog
    if qi in (3, 8, 12):
        return (1, 2)
    return (2,)


def _emit(ctx, tc):
    nc = tc.nc

    x8_d = nc.dram_tensor("x8", [P, 2, S], FP8, kind="ExternalInput")
    xte_d = nc.dram_tensor("xte", [P, S // P, E], BF16, kind="ExternalInput")
    m8_d = nc.dram_tensor("m8", [P, 2, E], FP8, kind="ExternalInput")
    wvb_d = nc.dram_tensor("wvb", [P, 2, E], BF16, kind="ExternalInput")
    u16_d = nc.dram_tensor("u16c", [P, 2], F32, kind="ExternalInput")
    out_d = nc.dram_tensor("out", [P, 2], F32, kind="ExternalOutput")

    const = ctx.enter_context(tc.tile_pool(name="const", bufs=1))
    epool = ctx.enter_context(tc.tile_pool(name="epool", bufs=COLSUM_LAG + 1))
    rsp = ctx.enter_context(tc.tile_pool(name="rsp", bufs=COLSUM_LAG + 2))
    pp = ctx.enter_context(tc.tile_pool(name="pp", bufs=2, space="PSUM"))
    wp = ctx.enter_context(tc.tile_pool(name="wp", bufs=1, space="PSUM"))

    # ---- small loads first so the q' projection can start immediately.
    m8 = const.tile([P, 2, E], FP8, name="m8")
    u16 = const.tile([P, 2], F32, name="u16")

    # x^T in fp8, [e-part, e-chunk-plane, t] -- DoubleRow rhs layout.
    # x8_0 issues before m8/u16: it is the largest of the three blockers of
    # the first matmul.
    bounds = [(0, 512), (512, 512), (1024, 1024), (2048, 1024), (3072, 1024)]
    x8c = [None] * len(bounds)
    for i, (c0, w) in enumerate(bounds):
        t = const.tile([P, 2, w], FP8, name=f"x8_{i}", tag=f"x8_{i}")
        x8c[i] = t
    nc.sync.dma_start(out=x8c[0], in_=x8_d[:, :, 0:512])
    nc.sync.dma_start(out=m8, in_=m8_d[:, :, :])
    nc.sync.dma_start(out=u16, in_=u16_d[:, :])
    for i, (c0, w) in enumerate(bounds):
        if i == 0:
            continue
        eng = nc.scalar if i % 2 else nc.sync
        eng.dma_start(out=x8c[i], in_=x8_d[:, :, c0 : c0 + w])

    def x8s(t0, width):
        for i, (c0, cw) in enumerate(bounds):
            if c0 <= t0 and t0 + width <= c0 + cw:
                return x8c[i][:, :, t0 - c0 : t0 - c0 + width]
        raise AssertionError(f"x8 slice [{t0}, {t0+width}) crosses chunk bounds")

    # x rows in bf16, [t-part, t-tile, e] -- final matvec rhs (tail only).
    # xte rides the sync queue LAST: it is 2MB and would block the scalar
    # (ACT) queue ahead of the q' casts; sync has nothing else to do.
    xte = const.tile([P, S // P, E], BF16, name="xte")
    nc.sync.dma_start(out=xte, in_=xte_d[:, :, :])
    wvb = const.tile([P, 2, E], BF16, name="wvb")
    nc.scalar.dma_start(out=wvb, in_=wvb_d[:, :, :])

    identity = const.tile([P, P], F32, name="identity")
    from concourse.masks import make_identity

    make_identity(nc, identity)
    negc = const.tile([P, 1], F32, name="negc")
    nc.vector.memset(negc, -CSHIFT)

    # ---- q' projection: q'16^T = M16^T @ x^T + u16 (DoubleRow, fp8 out).
    # One SBUF tile per span, aligned so q-tile qi's lhsT slice lives in a
    # single span tile: Tile's dep tracking then lets early q-tiles start as
    # soon as THEIR span's cast lands instead of waiting for all of q'.
    # Casts for the early spans ride the (startup-idle) ACT engine; the last
    # two go to DVE to keep ACT's steady-state load down.
    qspans = [(0, 128), (128, 384), (512, 512), (1024, 512), (1536, 512)]
    q8sp = [
        const.tile([P, 2, width], FP8, name=f"q8_{c0}", tag=f"q8_{c0}")
        for c0, width in qspans
    ]

    def emit_qproj_span(si):
        c0, width = qspans[si]
        for eo in range(2):
            ps = pp.tile([P, 1536], F32, tag="ps", name=f"ps_q{eo}_{c0}")
            nc.tensor.matmul(
                ps[:, 0:width],
                m8[:, :, eo * P : (eo + 1) * P],
                x8s(c0, width),
                start=True,
                stop=True,
                perf_mode=DR,
            )
            if si < 3:
                nc.scalar.activation(
                    out=q8sp[si][:, eo, :],
                    in_=ps[:, 0:width],
                    func=mybir.ActivationFunctionType.Identity,
                    bias=u16[:, eo : eo + 1],
                    scale=1.0,
                )
            else:
                nc.vector.tensor_scalar(
                    out=q8sp[si][:, eo, :],
                    in0=ps[:, 0:width],
                    scalar1=u16[:, eo : eo + 1],
                    scalar2=None,
                    op0=mybir.AluOpType.add,
                )

    # span 0 first; spans 1-4 are interleaved into the score loop below so
    # q-tile 0's scores don't queue behind the whole (cast-paced) projection
    # on the in-order PE.
    emit_qproj_span(0)

    def q8_lhsT(qi):
        q0 = qi * P
        for si, (c0, width) in enumerate(qspans):
            if c0 <= q0 and q0 + P <= c0 + width:
                return q8sp[si][:, :, q0 - c0 : q0 - c0 + P]
        raise AssertionError(f"q-tile {qi} crosses q8 span bounds")

    # ---- w accumulators: key chunk m (512 wide) -> tile m//4, strip 32*(m%4)
    w_ps = [wp.tile([P, 512], F32, tag=f"w{i}", name=f"w_ps{i}") for i in range(2)]
    for i in range(2):
        nc.vector.memset(w_ps[i], 0.0)

    def emit_scores(qi):
        Es = epool.tile([P, S], BF16, tag="E", name=f"E{qi}")
        dve = _dve_parts(qi)
        rs_a = rsp.tile([P, 3], F32, tag="rs", name=f"rs{qi}")
        rs_d = rsp.tile([P, 8], BF16, tag="rsd", name=f"rsd{qi}") if dve else None
        # 512-wide sub-part slots in rs_d, ordered so every write starts
        # 4B-aligned (2x DVE mode needs it): the 1024 chunk first, then 1536.
        dslot = {2: 0, 1: 2} if len(dve) == 2 else {c: 0 for c in dve}
        ndt = sum(CHUNKS[c][1] // 512 for c in dve)
        ei16 = Es.bitcast(I16)
        na = 0
        for c, (c0, width) in enumerate(CHUNKS):
            ps = pp.tile([P, 1536], F32, tag="ps", name=f"ps_s{qi}_{c}")
            for h in range(width // 512):
                t0 = c0 + h * 512
                nc.tensor.matmul(
                    ps[:, h * 512 : (h + 1) * 512],
                    q8_lhsT(qi),
                    x8s(t0, 512),
                    start=True,
                    stop=True,
                    perf_mode=DR,
                )
            sl = slice(c0, c0 + width)
            if c in dve:
                nc.vector.tensor_scalar(
                    out=ei16[:, sl],
                    in0=ps[:, 0:width],
                    scalar1=float(A2),
                    scalar2=float(B2),
                    op0=mybir.AluOpType.mult,
                    op1=mybir.AluOpType.add,
                )
                # 3D view [p, parts, 512], reduce innermost only: multi-element
                # bf16 output keeps the op eligible for the 2x perf mode.
                npt = width // 512
                s0 = dslot[c]
                with nc.allow_low_precision(reason="bf16 rowsum partial, 0.4% noise"):
                    nc.vector.reduce_sum(
                        out=rs_d[:, s0 : s0 + npt],
                        in_=Es[:, sl].rearrange("p (a b) -> p a b", b=512),
                        axis=mybir.AxisListType.X,
                    )
            else:
                nc.scalar.activation(
                    out=Es[:, sl],
                    in_=ps[:, 0:width],
                    func=mybir.ActivationFunctionType.Exp,
                    scale=1.0 / 256.0,
                    bias=negc,
                    accum_out=rs_a[:, na : na + 1],
                )
                na += 1
        ra = rsp.tile([P, 1], F32, tag="ra", name=f"ra{qi}")
        nc.vector.reduce_sum(out=ra, in_=rs_a[:, 0:na], axis=mybir.AxisListType.X)
        if ndt:
            rd = rsp.tile([P, 1], F32, tag="rd", name=f"rd{qi}")
            nc.vector.reduce_sum(out=rd, in_=rs_d[:, 0:ndt], axis=mybir.AxisListType.X)
            rn = rsp.tile([P, 1], F32, tag="rn", name=f"rn{qi}")
            nc.vector.tensor_add(rn, ra, rd)
            ra = rn
        recb = rsp.tile([P, 1], BF16, tag="recb", name=f"recb{qi}")
        with nc.allow_low_precision(reason="bf16 softmax reciprocal, as before"):
            nc.vector.reciprocal(out=recb, in_=ra)
        return Es, recb

    def emit_colsum(qi, Es, recb):
        for m in range(8):
            wt, strip = w_ps[m // 4], 32 * (m % 4)
            nc.tensor.matmul(
                wt[strip : strip + 1, :],
                recb,
                Es[:, m * 512 : (m + 1) * 512],
                start=(qi == 0),
                stop=(qi == QTILES - 1),
                tile_position=(0, strip),
            )

    pending = {}
    for qi in range(QTILES):
        pending[qi] = emit_scores(qi)
        if qi < 4:
            emit_qproj_span(qi + 1)   # ready well before q-tile 4*(si) needs it
        if qi - COLSUM_LAG in pending:
            emit_colsum(qi - COLSUM_LAG, *pending.pop(qi - COLSUM_LAG))
    for qi in sorted(pending):
        emit_colsum(qi, *pending.pop(qi))

    # ---- tail: w strips -> SBUF, PE-transpose to w^T, matvec against x,
    # then apply Wv to the pooled vector on-device.
    # w_sb_f[strip 32k, 512a + u] = w[t = 2048a + 512k + u]
    w_sb_f = const.tile([P, 1024], F32, name="w_sb_f")
    nc.vector.tensor_copy(out=w_sb_f[:, 0:512], in_=w_ps[0])
    nc.scalar.copy(out=w_sb_f[:, 512:1024], in_=w_ps[1])
    # transpose targets alternate between two psum tiles so each strided
    # cast (a read of tile c%2) doesn't serialize against the NEXT transpose
    # (a write to tile (c+1)%2) via tile-granular WAR tracking.
    wt_sb = const.tile([P, 32], BF16, name="wt_sb")
    tpa = pp.tile([P, 1536], F32, tag="ps", name="tp_a")
    tpb = pp.tile([P, 1536], F32, tag="ps", name="tp_b")
    for c in range(8):
        tp = (tpa, tpb)[c % 2]
        off = (c // 2) * P
        nc.tensor.transpose(
            out=tp[:, off : off + P],
            in_=w_sb_f[:, c * P : (c + 1) * P],
            identity=identity,
        )
        src = bass.AP(
            tensor=tp.tensor, offset=tp.offset + off, ap=[tp.ap[0], [32, 4]]
        )
        nc.vector.tensor_copy(out=wt_sb[:, c * 4 : (c + 1) * 4], in_=src)
    # pooled-x partials: 4 concurrent column strips at partitions {0,32,64,96}
    fin = wp.tile([P, 512], F32, tag="w0", name="fin")
    nc.vector.memset(fin[:, 0:E], 0.0)
    for col in range(32):
        c, k = divmod(col, 4)
        vidx = 16 * (c // 4) + 4 * k + (c % 4)
        strip = 32 * (col % 4)
        nc.tensor.matmul(
            fin[strip : strip + 1, 0:E],
            wt_sb[:, col : col + 1],
            xte[:, vidx, :],
            start=(col < 4),
            stop=(col >= 28),
            tile_position=(0, strip),
        )
    strips_sb = const.tile([P, E], F32, name="strips_sb")
    nc.vector.tensor_copy(out=strips_sb, in_=fin[:, 0:E])
    # transpose the 4 strip partials to [e-part, strip] and apply Wv
    ss4 = const.tile([P, 2, 4], BF16, name="ss4")
    for ei in range(2):
        tpe = pp.tile([P, 1536], F32, tag="ps", name=f"tpe{ei}")
        nc.tensor.transpose(
            out=tpe[:, 0:P], in_=strips_sb[:, ei * P : (ei + 1) * P], identity=identity
        )
        src = bass.AP(tensor=tpe.tensor, offset=tpe.offset, ap=[tpe.ap[0], [32, 4]])
        nc.vector.tensor_copy(out=ss4[:, ei, :], in_=src)
    pf = wp.tile([P, 512], F32, tag="w1", name="pf")
    for eo in range(2):
        for ei in range(2):
            nc.tensor.matmul(
                pf[:, eo * 4 : (eo + 1) * 4],
                wvb[:, ei, eo * P : (eo + 1) * P],
                ss4[:, ei, :],
                start=(ei == 0),
                stop=(ei == 1),
            )
    po = const.tile([P, 2], F32, name="po")
    for eo in range(2):
        nc.vector.reduce_sum(
            out=po[:, eo : eo + 1],
            in_=pf[:, eo * 4 : (eo + 1) * 4],
            axis=mybir.AxisListType.X,
        )
    nc.sync.dma_start(out=out_d[:, :], in_=po)


_NC_CACHE = None


def _build_nc():
    global _NC_CACHE
    if _NC_CACHE is None:
        from contextlib import ExitStack

        nc = bacc.Bacc("TRN2", target_bir_lowering=False, debug=False)
        with tile.TileContext(nc) as tc, ExitStack() as ctx:
            _emit(ctx, tc)
        nc.compile()
        _NC_CACHE = nc
    return _NC_CACHE


def _in_maps(inputs):
    import ml_dtypes

    bf16 = ml_dtypes.bfloat16
    f8 = ml_dtypes.float8_e4m3

    def to8(a):
        return np.clip(a, -240.0, 240.0).astype(f8)

    x = np.asarray(inputs["x"], dtype=np.float32)
    Wq = np.asarray(inputs["Wq"], dtype=np.float64)
    Wk = np.asarray(inputs["Wk"], dtype=np.float64)
    Wv = np.asarray(inputs["Wv"], dtype=np.float32)
    bq = np.asarray(inputs["bq"], dtype=np.float64)

    M16 = (16.0 * (Wq @ Wk.T)).astype(np.float32)
    u16 = (16.0 * (Wk @ bq)).astype(np.float32)
    m8 = np.ascontiguousarray(to8(M16).reshape(2, P, E).transpose(1, 0, 2))
    u16c = np.ascontiguousarray(u16.reshape(2, P).T)
    wvb = np.ascontiguousarray(Wv.astype(bf16).reshape(2, P, E).transpose(1, 0, 2))

    maps = []
    for c in range(N_CORES):
        b, h = divmod(c, 2)
        xr = np.roll(x[b], -h * HALF, axis=0)
        x8 = np.ascontiguousarray(
            to8(xr.T).reshape(2, P, S).transpose(1, 0, 2)
        )
        xte = np.ascontiguousarray(
            xr.astype(bf16).reshape(S // P, P, E).transpose(1, 0, 2)
        )
        maps.append({"x8": x8, "xte": xte, "m8": m8, "wvb": wvb, "u16c": u16c})
    return maps


def _combine(results, inputs):
    bv = np.asarray(inputs["bv"], dtype=np.float32).reshape(E)
    pooled = [np.asarray(r["out"], np.float32).T.reshape(E) for r in results]
    out = np.stack(
        [(pooled[2 * b] + pooled[2 * b + 1]) / S + bv for b in range(B)]
    )[:, None, :]
    return out.astype(np.float32)


def kernel(**inputs):
    from concourse.bass_utils import run_bass_kernel_spmd

    nc = _build_nc()
    res = run_bass_kernel_spmd(nc, _in_maps(inputs), core_ids=list(range(N_CORES)))
    return _combine(res.results, inputs)


# revision 33
# speedup vs baseline: 1.0701x; 1.0701x over previous
"""Attention-pooling Trainium2 kernel (fp8 DoubleRow + split-engine exp).

Problem: out = mean_s(softmax((x@Wq+bq)(x@Wk+bk)^T / sqrt(E)) @ (x@Wv+bv))
with x [4, 4096, 256], output [4, 1, 256].

Math restructuring (exact up to fp reassociation):
  * mean_s(dist @ V) = (colsum(dist)/S) @ V  -- the second S x S matmul
    collapses to a length-S vector "w" and one matvec.
  * K bias drops (row-constant in scores); V bias folds to host "+bv".
  * Q/K projections fold into ONE projection: scores = x M x^T + u^T x^T
    with M = Wq Wk^T, u = Wk bq (host-computed E x E / E-sized weight prep).
    So the device never computes K.
  * Wv moves to the END: pooled = (w @ x) @ Wv -- the V projection
    (S x E x E) becomes an E x E matmul on a [1, E] vector.
  * Scores run in fp8(e4m3) with DoubleRow perf mode: the E=256
    contraction happens in ONE PE pass at 2x bf16 rate. M is pre-scaled
    by 16 host-side so fp8 operands sit in their sweet spot; the exp
    applies scale 1/256 and a constant -2 shift to keep exp outputs in
    range. Numerics validated in simulation: rel_err ~0.009 vs 2e-2 gate.
  * exp is split across engines: ACT computes true exp (with accum_out
    row-sums); DVE computes a Schraudolph-style exp -- one tensor_scalar
    (score*A + B) -> int16, whose bit pattern IS the bf16 exp
    approximation (+-3.5% sawtooth, washes out in the pooled mean).

Sharding: 8 cores = 4 batches x 2 query-row halves; x arrives rolled so
each core's 2048 query rows are columns 0:2047 (permutation-invariant
for the pooled result). Host sums the two halves per batch, /S, +bv.
"""

import numpy as np

import concourse.bass as bass  # noqa: F401
import concourse.mybir as mybir
import concourse.tile as tile
from concourse import bacc

B, S, E = 4, 4096, 256
HALF = S // 2          # query rows per core
P = 128
N_CORES = 8
QTILES = HALF // P     # 16
F32 = mybir.dt.float32
BF16 = mybir.dt.bfloat16
FP8 = mybir.dt.float8e4
I16 = mybir.dt.int16
DR = mybir.MatmulPerfMode.DoubleRow

CSHIFT = 2.0                       # exp(score - CSHIFT): keeps e4m3/bf16 in range
A_SCH = 128.0 / np.log(2.0)        # bf16 Schraudolph slope (per unit exp arg)
A2 = A_SCH / 256.0                 # folded score scale 1/256
B2 = (127 * 128 - 5.5) - CSHIFT * A_SCH
COLSUM_LAG = 2
# per-qtile chunking of the 4096 keys; chunk index -> (start, width).
# ACT chunks run true exp with accum_out rowsums; DVE chunks run the
# Schraudolph tensor_scalar with a bf16 reduce for their rowsum.
CHUNKS = [(0, 1536), (1536, 1536), (3072, 1024)]


def _dve_parts(qi):
    if qi in (14, 15):
        return ()          # tail qtiles all-ACT so recb isn't on the DVE backlog
    if qi in (3, 8, 12):
        return (1, 2)
    return (2,)


def _emit(ctx, tc):
    nc = tc.nc

    x8_d = nc.dram_tensor("x8", [P, 2, S], FP8, kind="ExternalInput")
    xte_d = nc.dram_tensor("xte", [P, S // P, E], BF16, kind="ExternalInput")
    m8_d = nc.dram_tensor("m8", [P, 2, E], FP8, kind="ExternalInput")
    wvb_d = nc.dram_tensor("wvb", [P, 2, E], BF16, kind="ExternalInput")
    u16_d = nc.dram_tensor("u16c", [P, 2], F32, kind="ExternalInput")
    out_d = nc.dram_tensor("out", [P, 2], F32, kind="ExternalOutput")

    const = ctx.enter_context(tc.tile_pool(name="const", bufs=1))
    epool = ctx.enter_context(tc.tile_pool(name="epool", bufs=COLSUM_LAG + 1))
    rsp = ctx.enter_context(tc.tile_pool(name="rsp", bufs=COLSUM_LAG + 2))
    pp = ctx.enter_context(tc.tile_pool(name="pp", bufs=2, space="PSUM"))
    wp = ctx.enter_context(tc.tile_pool(name="wp", bufs=1, space="PSUM"))

    # ---- small loads first so the q' projection can start immediately.
    m8 = const.tile([P, 2, E], FP8, name="m8")
    u16 = const.tile([P, 2], F32, name="u16")

    # x^T in fp8, [e-part, e-chunk-plane, t] -- DoubleRow rhs layout.
    # x8_0 issues before m8/u16: it is the largest of the three blockers of
    # the first matmul.
    bounds = [(0, 512), (512, 512), (1024, 1024), (2048, 1024), (3072, 1024)]
    x8c = [None] * len(bounds)
    for i, (c0, w) in enumerate(bounds):
        t = const.tile([P, 2, w], FP8, name=f"x8_{i}", tag=f"x8_{i}")
        x8c[i] = t
    nc.sync.dma_start(out=x8c[0], in_=x8_d[:, :, 0:512])
    nc.sync.dma_start(out=m8, in_=m8_d[:, :, :])
    nc.sync.dma_start(out=u16, in_=u16_d[:, :])
    for i, (c0, w) in enumerate(bounds):
        if i == 0:
            continue
        eng = nc.scalar if i % 2 else nc.sync
        eng.dma_start(out=x8c[i], in_=x8_d[:, :, c0 : c0 + w])

    def x8s(t0, width):
        for i, (c0, cw) in enumerate(bounds):
            if c0 <= t0 and t0 + width <= c0 + cw:
                return x8c[i][:, :, t0 - c0 : t0 - c0 + width]
        raise AssertionError(f"x8 slice [{t0}, {t0+width}) crosses chunk bounds")

    # x rows in bf16, [t-part, t-tile, e] -- final matvec rhs (tail only).
    # xte rides the sync queue LAST: it is 2MB and would block the scalar
    # (ACT) queue ahead of the q' casts; sync has nothing else to do.
    xte = const.tile([P, S // P, E], BF16, name="xte")
    nc.sync.dma_start(out=xte, in_=xte_d[:, :, :])
    wvb = const.tile([P, 2, E], BF16, name="wvb")
    nc.scalar.dma_start(out=wvb, in_=wvb_d[:, :, :])

    identity = const.tile([P, P], F32, name="identity")
    from concourse.masks import make_identity

    make_identity(nc, identity)
    negc = const.tile([P, 1], F32, name="negc")
    nc.vector.memset(negc, -CSHIFT)

    # ---- q' projection: q'16^T = M16^T @ x^T + u16 (DoubleRow, fp8 out).
    # One SBUF tile per span, aligned so q-tile qi's lhsT slice lives in a
    # single span tile: Tile's dep tracking then lets early q-tiles start as
    # soon as THEIR span's cast lands instead of waiting for all of q'.
    # Casts for the early spans ride the (startup-idle) ACT engine; the last
    # two go to DVE to keep ACT's steady-state load down.
    qspans = [(0, 128), (128, 384), (512, 512), (1024, 512), (1536, 512)]
    q8sp = [
        const.tile([P, 2, width], FP8, name=f"q8_{c0}", tag=f"q8_{c0}")
        for c0, width in qspans
    ]
    for si, (c0, width) in enumerate(qspans):
        for eo in range(2):
            ps = pp.tile([P, 1536], F32, tag="ps", name=f"ps_q{eo}_{c0}")
            nc.tensor.matmul(
                ps[:, 0:width],
                m8[:, :, eo * P : (eo + 1) * P],
                x8s(c0, width),
                start=True,
                stop=True,
                perf_mode=DR,
            )
            if si < 3:
                nc.scalar.activation(
                    out=q8sp[si][:, eo, :],
                    in_=ps[:, 0:width],
                    func=mybir.ActivationFunctionType.Identity,
                    bias=u16[:, eo : eo + 1],
                    scale=1.0,
                )
            else:
                nc.vector.tensor_scalar(
                    out=q8sp[si][:, eo, :],
                    in0=ps[:, 0:width],
                    scalar1=u16[:, eo : eo + 1],
                    scalar2=None,
                    op0=mybir.AluOpType.add,
                )

    def q8_lhsT(qi):
        q0 = qi * P
        for si, (c0, width) in enumerate(qspans):
            if c0 <= q0 and q0 + P <= c0 + width:
                return q8sp[si][:, :, q0 - c0 : q0 - c0 + P]
        raise AssertionError(f"q-tile {qi} crosses q8 span bounds")

    # ---- w accumulators: key chunk m (512 wide) -> tile m//4, strip 32*(m%4)
    w_ps = [wp.tile([P, 512], F32, tag=f"w{i}", name=f"w_ps{i}") for i in range(2)]
    for i in range(2):
        nc.vector.memset(w_ps[i], 0.0)

    def emit_scores(qi):
        Es = epool.tile([P, S], BF16, tag="E", name=f"E{qi}")
        dve = _dve_parts(qi)
        rs_a = rsp.tile([P, 3], F32, tag="rs", name=f"rs{qi}")
        rs_d = rsp.tile([P, 8], BF16, tag="rsd", name=f"rsd{qi}") if dve else None
        # 512-wide sub-part slots in rs_d, ordered so every write starts
        # 4B-aligned (2x DVE mode needs it): the 1024 chunk first, then 1536.
        dslot = {2: 0, 1: 2} if len(dve) == 2 else {c: 0 for c in dve}
        ndt = sum(CHUNKS[c][1] // 512 for c in dve)
        ei16 = Es.bitcast(I16)
        na = 0
        for c, (c0, width) in enumerate(CHUNKS):
            ps = pp.tile([P, 1536], F32, tag="ps", name=f"ps_s{qi}_{c}")
            for h in range(width // 512):
                t0 = c0 + h * 512
                nc.tensor.matmul(
                    ps[:, h * 512 : (h + 1) * 512],
                    q8_lhsT(qi),
                    x8s(t0, 512),
                    start=True,
                    stop=True,
                    perf_mode=DR,
                )
            sl = slice(c0, c0 + width)
            if c in dve:
                nc.vector.tensor_scalar(
                    out=ei16[:, sl],
                    in0=ps[:, 0:width],
                    scalar1=float(A2),
                    scalar2=float(B2),
                    op0=mybir.AluOpType.mult,
                    op1=mybir.AluOpType.add,
                )
                # 3D view [p, parts, 512], reduce innermost only: multi-element
                # bf16 output keeps the op eligible for the 2x perf mode.
                npt = width // 512
                s0 = dslot[c]
                with nc.allow_low_precision(reason="bf16 rowsum partial, 0.4% noise"):
                    nc.vector.reduce_sum(
                        out=rs_d[:, s0 : s0 + npt],
                        in_=Es[:, sl].rearrange("p (a b) -> p a b", b=512),
                        axis=mybir.AxisListType.X,
                    )
            else:
                nc.scalar.activation(
                    out=Es[:, sl],
                    in_=ps[:, 0:width],
                    func=mybir.ActivationFunctionType.Exp,
                    scale=1.0 / 256.0,
                    bias=negc,
                    accum_out=rs_a[:, na : na + 1],
                )
                na += 1
        ra = rsp.tile([P, 1], F32, tag="ra", name=f"ra{qi}")
        nc.vector.reduce_sum(out=ra, in_=rs_a[:, 0:na], axis=mybir.AxisListType.X)
        if ndt:
            rd = rsp.tile([P, 1], F32, tag="rd", name=f"rd{qi}")
            nc.vector.reduce_sum(out=rd, in_=rs_d[:, 0:ndt], axis=mybir.AxisListType.X)
            rn = rsp.tile([P, 1], F32, tag="rn", name=f"rn{qi}")
            nc.vector.tensor_add(rn, ra, rd)
            ra = rn
        recb = rsp.tile([P, 1], BF16, tag="recb", name=f"recb{qi}")
        with nc.allow_low_precision(reason="bf16 softmax reciprocal, as before"):
            nc.vector.reciprocal(out=recb, in_=ra)
        return Es, recb

    def emit_colsum(qi, Es, recb):
        for m in range(8):
            wt, strip = w_ps[m // 4], 32 * (m % 4)
            nc.tensor.matmul(
                wt[strip : strip + 1, :],
                recb,
                Es[:, m * 512 : (m + 1) * 512],
                start=(qi == 0),
                stop=(qi == QTILES - 1),
                tile_position=(0, strip),
            )

    pending = {}
    for qi in range(QTILES):
        pending[qi] = emit_scores(qi)
        if qi - COLSUM_LAG in pending:
            emit_colsum(qi - COLSUM_LAG, *pending.pop(qi - COLSUM_LAG))
    for qi in sorted(pending):
        emit_colsum(qi, *pending.pop(qi))

    # ---- tail: w strips -> SBUF, PE-transpose to w^T, matvec against x,
    # then apply Wv to the pooled vector on-device.
    # w_sb_f[strip 32k, 512a + u] = w[t = 2048a + 512k + u]
    w_sb_f = const.tile([P, 1024], F32, name="w_sb_f")
    nc.vector.tensor_copy(out=w_sb_f[:, 0:512], in_=w_ps[0])
    nc.scalar.copy(out=w_sb_f[:, 512:1024], in_=w_ps[1])
    # transpose targets alternate between two psum tiles so each strided
    # cast (a read of tile c%2) doesn't serialize against the NEXT transpose
    # (a write to tile (c+1)%2) via tile-granular WAR tracking.
    wt_sb = const.tile([P, 32], BF16, name="wt_sb")
    tpa = pp.tile([P, 1536], F32, tag="ps", name="tp_a")
    tpb = pp.tile([P, 1536], F32, tag="ps", name="tp_b")
    for c in range(8):
        tp = (tpa, tpb)[c % 2]
        off = (c // 2) * P
        nc.tensor.transpose(
            out=tp[:, off : off + P],
            in_=w_sb_f[:, c * P : (c + 1) * P],
            identity=identity,
        )
        src = bass.AP(
            tensor=tp.tensor, offset=tp.offset + off, ap=[tp.ap[0], [32, 4]]
        )
        nc.vector.tensor_copy(out=wt_sb[:, c * 4 : (c + 1) * 4], in_=src)
    # pooled-x partials: 4 concurrent column strips at partitions {0,32,64,96}
    fin = wp.tile([P, 512], F32, tag="w0", name="fin")
    nc.vector.memset(fin[:, 0:E], 0.0)
    for col in range(32):
        c, k = divmod(col, 4)
        vidx = 16 * (c // 4) + 4 * k + (c % 4)
        strip = 32 * (col % 4)
        nc.tensor.matmul(
            fin[strip : strip + 1, 0:E],
            wt_sb[:, col : col + 1],
            xte[:, vidx, :],
            start=(col < 4),
            stop=(col >= 28),
            tile_position=(0, strip),
        )
    strips_sb = const.tile([P, E], F32, name="strips_sb")
    nc.vector.tensor_copy(out=strips_sb, in_=fin[:, 0:E])
    # transpose the 4 strip partials to [e-part, strip] and apply Wv
    ss4 = const.tile([P, 2, 4], BF16, name="ss4")
    for ei in range(2):
        tpe = pp.tile([P, 1536], F32, tag="ps", name=f"tpe{ei}")
        nc.tensor.transpose(
            out=tpe[:, 0:P], in_=strips_sb[:, ei * P : (ei + 1) * P], identity=identity
        )
        src = bass.AP(tensor=tpe.tensor, offset=tpe.offset, ap=[tpe.ap[0], [32, 4]])
        nc.vector.tensor_copy(out=ss4[:, ei, :], in_=src)
    pf = wp.tile([P, 512], F32, tag="w1", name="pf")
    for eo in range(2):
        for ei in range(2):
            nc.tensor.matmul(
                pf[:, eo * 4 : (eo + 1) * 4],
                wvb[:, ei, eo * P : (eo + 1) * P],
                ss4[:, ei, :],
                start=(ei == 0),
                stop=(ei == 1),
            )
    po = const.tile([P, 2], F32, name="po")
    for eo in range(2):
        nc.vector.reduce_sum(
            out=po[:, eo : eo + 1],
            in_=pf[:, eo * 4 : (eo + 1) * 4],
            axis=mybir.AxisListType.X,
        )
    nc.sync.dma_start(out=out_d[:, :], in_=po)


_NC_CACHE = None


def _build_nc():
    global _NC_CACHE
    if _NC_CACHE is None:
        from contextlib import ExitStack

        nc = bacc.Bacc("TRN2", target_bir_lowering=False, debug=False)
        with tile.TileContext(nc) as tc, ExitStack() as ctx:
            _emit(ctx, tc)
        nc.compile()
        _NC_CACHE = nc
    return _NC_CACHE


def _in_maps(inputs):
    import ml_dtypes

    bf16 = ml_dtypes.bfloat16
    f8 = ml_dtypes.float8_e4m3

    def to8(a):
        return np.clip(a, -240.0, 240.0).astype(f8)

    x = np.asarray(inputs["x"], dtype=np.float32)
    Wq = np.asarray(inputs["Wq"], dtype=np.float64)
    Wk = np.asarray(inputs["Wk"], dtype=np.float64)
    Wv = np.asarray(inputs["Wv"], dtype=np.float32)
    bq = np.asarray(inputs["bq"], dtype=np.float64)

    M16 = (16.0 * (Wq @ Wk.T)).astype(np.float32)
    u16 = (16.0 * (Wk @ bq)).astype(np.float32)
    m8 = np.ascontiguousarray(to8(M16).reshape(2, P, E).transpose(1, 0, 2))
    u16c = np.ascontiguousarray(u16.reshape(2, P).T)
    wvb = np.ascontiguousarray(Wv.astype(bf16).reshape(2, P, E).transpose(1, 0, 2))

    maps = []
    for c in range(N_CORES):
        b, h = divmod(c, 2)
        xr = np.roll(x[b], -h * HALF, axis=0)
        x8 = np.ascontiguousarray(
            to8(xr.T).reshape(2, P, S).transpose(1, 0, 2)
        )
        xte = np.ascontiguousarray(
            xr.astype(bf16).reshape(S // P, P, E).transpose(1, 0, 2)
        )
        maps.append({"x8": x8, "xte": xte, "m8": m8, "wvb": wvb, "u16c": u16c})
    return maps


def _combine(results, inputs):
    bv = np.asarray(inputs["bv"], dtype=np.float32).reshape(E)
    pooled = [np.asarray(r["out"], np.float32).T.reshape(E) for r in results]
    out = np.stack(
        [(pooled[2 * b] + pooled[2 * b + 1]) / S + bv for b in range(B)]
    )[:, None, :]
    return out.astype(np.float32)


def kernel(**inputs):
    from concourse.bass_utils import run_bass_kernel_spmd

    nc = _build_nc()
    res = run_bass_kernel_spmd(nc, _in_maps(inputs), core_ids=list(range(N_CORES)))
    return _combine(res.results, inputs)


# revision 35
# speedup vs baseline: 1.0889x; 1.0176x over previous
"""Attention-pooling Trainium2 kernel (fp8 DoubleRow + split-engine exp).

Problem: out = mean_s(softmax((x@Wq+bq)(x@Wk+bk)^T / sqrt(E)) @ (x@Wv+bv))
with x [4, 4096, 256], output [4, 1, 256].

Math restructuring (exact up to fp reassociation):
  * mean_s(dist @ V) = (colsum(dist)/S) @ V  -- the second S x S matmul
    collapses to a length-S vector "w" and one matvec.
  * K bias drops (row-constant in scores); V bias folds to host "+bv".
  * Q/K projections fold into ONE projection: scores = x M x^T + u^T x^T
    with M = Wq Wk^T, u = Wk bq (host-computed E x E / E-sized weight prep).
    So the device never computes K.
  * Wv moves to the END: pooled = (w @ x) @ Wv -- the V projection
    (S x E x E) becomes an E x E matmul on a [1, E] vector.
  * Scores run in fp8(e4m3) with DoubleRow perf mode: the E=256
    contraction happens in ONE PE pass at 2x bf16 rate. M is pre-scaled
    by 16 host-side so fp8 operands sit in their sweet spot; the exp
    applies scale 1/256 and a constant -2 shift to keep exp outputs in
    range. Numerics validated in simulation: rel_err ~0.009 vs 2e-2 gate.
  * exp is split across engines: ACT computes true exp (with accum_out
    row-sums); DVE computes a Schraudolph-style exp -- one tensor_scalar
    (score*A + B) -> int16, whose bit pattern IS the bf16 exp
    approximation (+-3.5% sawtooth, washes out in the pooled mean).

Sharding: 8 cores = 4 batches x 2 query-row halves; x arrives rolled so
each core's 2048 query rows are columns 0:2047 (permutation-invariant
for the pooled result). Host sums the two halves per batch, /S, +bv.
"""

import numpy as np

import concourse.bass as bass  # noqa: F401
import concourse.mybir as mybir
import concourse.tile as tile
from concourse import bacc

B, S, E = 4, 4096, 256
HALF = S // 2          # query rows per core
P = 128
N_CORES = 8
QTILES = HALF // P     # 16
F32 = mybir.dt.float32
BF16 = mybir.dt.bfloat16
FP8 = mybir.dt.float8e4
I16 = mybir.dt.int16
DR = mybir.MatmulPerfMode.DoubleRow

CSHIFT = 2.0                       # exp(score - CSHIFT): keeps e4m3/bf16 in range
A_SCH = 128.0 / np.log(2.0)        # bf16 Schraudolph slope (per unit exp arg)
A2 = A_SCH / 256.0                 # folded score scale 1/256
B2 = (127 * 128 - 5.5) - CSHIFT * A_SCH
COLSUM_LAG = 2
# per-qtile chunking of the 4096 keys; chunk index -> (start, width).
# ACT chunks run true exp with accum_out rowsums; DVE chunks run the
# Schraudolph tensor_scalar with a bf16 reduce for their rowsum.
CHUNKS = [(0, 1536), (1536, 1536), (3072, 1024)]


def _dve_parts(qi):
    if qi in (14, 15):
        return ()          # tail qtiles all-ACT so recb isn't on the DVE backlog
    if qi in (3, 8, 12):
        return (1, 2)
    return (2,)


def _emit(ctx, tc):
    nc = tc.nc

    x8_d = nc.dram_tensor("x8", [P, 2, S], FP8, kind="ExternalInput")
    xte_d = nc.dram_tensor("xte", [P, S // P, E], BF16, kind="ExternalInput")
    m8_d = nc.dram_tensor("m8", [P, 2, E], FP8, kind="ExternalInput")
    wvb_d = nc.dram_tensor("wvb", [P, 2, E], BF16, kind="ExternalInput")
    u16_d = nc.dram_tensor("u16c", [P, 2], F32, kind="ExternalInput")
    out_d = nc.dram_tensor("out", [P, 2], F32, kind="ExternalOutput")

    const = ctx.enter_context(tc.tile_pool(name="const", bufs=1))
    epool = ctx.enter_context(tc.tile_pool(name="epool", bufs=COLSUM_LAG + 1))
    rsp = ctx.enter_context(tc.tile_pool(name="rsp", bufs=COLSUM_LAG + 2))
    pp = ctx.enter_context(tc.tile_pool(name="pp", bufs=2, space="PSUM"))
    wp = ctx.enter_context(tc.tile_pool(name="wp", bufs=1, space="PSUM"))

    # ---- small loads first so the q' projection can start immediately.
    m8 = const.tile([P, 2, E], FP8, name="m8")
    u16 = const.tile([P, 2], F32, name="u16")

    # x^T in fp8, [e-part, e-chunk-plane, t] -- DoubleRow rhs layout.
    # x8_0 issues before m8/u16: it is the largest of the three blockers of
    # the first matmul.
    bounds = [(0, 512), (512, 512), (1024, 1024), (2048, 1024), (3072, 1024)]
    x8c = [None] * len(bounds)
    for i, (c0, w) in enumerate(bounds):
        t = const.tile([P, 2, w], FP8, name=f"x8_{i}", tag=f"x8_{i}")
        x8c[i] = t
    nc.sync.dma_start(out=m8, in_=m8_d[:, :, :])
    nc.sync.dma_start(out=x8c[0], in_=x8_d[:, :, 0:512])
    nc.sync.dma_start(out=u16, in_=u16_d[:, :])
    for i, (c0, w) in enumerate(bounds):
        if i == 0:
            continue
        eng = nc.scalar if i % 2 else nc.sync
        eng.dma_start(out=x8c[i], in_=x8_d[:, :, c0 : c0 + w])

    def x8s(t0, width):
        for i, (c0, cw) in enumerate(bounds):
            if c0 <= t0 and t0 + width <= c0 + cw:
                return x8c[i][:, :, t0 - c0 : t0 - c0 + width]
        raise AssertionError(f"x8 slice [{t0}, {t0+width}) crosses chunk bounds")

    # x rows in bf16, [t-part, t-tile, e] -- final matvec rhs (tail only).
    # xte rides the sync queue LAST: it is 2MB and would block the scalar
    # (ACT) queue ahead of the q' casts; sync has nothing else to do.
    xte = const.tile([P, S // P, E], BF16, name="xte")
    nc.sync.dma_start(out=xte, in_=xte_d[:, :, :])
    wvb = const.tile([P, 2, E], BF16, name="wvb")
    nc.scalar.dma_start(out=wvb, in_=wvb_d[:, :, :])

    identity = const.tile([P, P], F32, name="identity")
    from concourse.masks import make_identity

    make_identity(nc, identity)
    negc = const.tile([P, 1], F32, name="negc")
    nc.vector.memset(negc, -CSHIFT)

    # ---- q' projection: q'16^T = M16^T @ x^T + u16 (DoubleRow, fp8 out).
    # One SBUF tile per span, aligned so q-tile qi's lhsT slice lives in a
    # single span tile: Tile's dep tracking then lets early q-tiles start as
    # soon as THEIR span's cast lands instead of waiting for all of q'.
    # Casts for the early spans ride the (startup-idle) ACT engine; the last
    # two go to DVE to keep ACT's steady-state load down.
    # Two 1024-wide spans: q-tile 0's scores still only wait for span 0's
    # cast, but the whole projection is 4 cast round-trips instead of 10.
    # All casts ride the (startup-idle) ACT engine, before any exp.
    qspans = [(0, 1024), (1024, 1024)]
    q8sp = [
        const.tile([P, 2, width], FP8, name=f"q8_{c0}", tag=f"q8_{c0}")
        for c0, width in qspans
    ]
    for si, (c0, width) in enumerate(qspans):
        for eo in range(2):
            ps = pp.tile([P, 1536], F32, tag="ps", name=f"ps_q{eo}_{c0}")
            for h in range(width // 512):
                nc.tensor.matmul(
                    ps[:, h * 512 : (h + 1) * 512],
                    m8[:, :, eo * P : (eo + 1) * P],
                    x8s(c0 + h * 512, 512),
                    start=True,
                    stop=True,
                    perf_mode=DR,
                )
            nc.scalar.activation(
                out=q8sp[si][:, eo, :],
                in_=ps[:, 0:width],
                func=mybir.ActivationFunctionType.Identity,
                bias=u16[:, eo : eo + 1],
                scale=1.0,
            )

    def q8_lhsT(qi):
        q0 = qi * P
        for si, (c0, width) in enumerate(qspans):
            if c0 <= q0 and q0 + P <= c0 + width:
                return q8sp[si][:, :, q0 - c0 : q0 - c0 + P]
        raise AssertionError(f"q-tile {qi} crosses q8 span bounds")

    # ---- w accumulators: key chunk m (512 wide) -> tile m//4, strip 32*(m%4)
    w_ps = [wp.tile([P, 512], F32, tag=f"w{i}", name=f"w_ps{i}") for i in range(2)]
    for i in range(2):
        nc.vector.memset(w_ps[i], 0.0)

    def emit_scores(qi):
        Es = epool.tile([P, S], BF16, tag="E", name=f"E{qi}")
        dve = _dve_parts(qi)
        rs_a = rsp.tile([P, 3], F32, tag="rs", name=f"rs{qi}")
        rs_d = rsp.tile([P, 8], BF16, tag="rsd", name=f"rsd{qi}") if dve else None
        # 512-wide sub-part slots in rs_d, ordered so every write starts
        # 4B-aligned (2x DVE mode needs it): the 1024 chunk first, then 1536.
        dslot = {2: 0, 1: 2} if len(dve) == 2 else {c: 0 for c in dve}
        ndt = sum(CHUNKS[c][1] // 512 for c in dve)
        ei16 = Es.bitcast(I16)
        na = 0
        for c, (c0, width) in enumerate(CHUNKS):
            ps = pp.tile([P, 1536], F32, tag="ps", name=f"ps_s{qi}_{c}")
            for h in range(width // 512):
                t0 = c0 + h * 512
                nc.tensor.matmul(
                    ps[:, h * 512 : (h + 1) * 512],
                    q8_lhsT(qi),
                    x8s(t0, 512),
                    start=True,
                    stop=True,
                    perf_mode=DR,
                )
            sl = slice(c0, c0 + width)
            if c in dve:
                nc.vector.tensor_scalar(
                    out=ei16[:, sl],
                    in0=ps[:, 0:width],
                    scalar1=float(A2),
                    scalar2=float(B2),
                    op0=mybir.AluOpType.mult,
                    op1=mybir.AluOpType.add,
                )
                # 3D view [p, parts, 512], reduce innermost only: multi-element
                # bf16 output keeps the op eligible for the 2x perf mode.
                npt = width // 512
                s0 = dslot[c]
                with nc.allow_low_precision(reason="bf16 rowsum partial, 0.4% noise"):
                    nc.vector.reduce_sum(
                        out=rs_d[:, s0 : s0 + npt],
                        in_=Es[:, sl].rearrange("p (a b) -> p a b", b=512),
                        axis=mybir.AxisListType.X,
                    )
            else:
                nc.scalar.activation(
                    out=Es[:, sl],
                    in_=ps[:, 0:width],
                    func=mybir.ActivationFunctionType.Exp,
                    scale=1.0 / 256.0,
                    bias=negc,
                    accum_out=rs_a[:, na : na + 1],
                )
                na += 1
        ra = rsp.tile([P, 1], F32, tag="ra", name=f"ra{qi}")
        nc.vector.reduce_sum(out=ra, in_=rs_a[:, 0:na], axis=mybir.AxisListType.X)
        if ndt:
            rd = rsp.tile([P, 1], F32, tag="rd", name=f"rd{qi}")
            nc.vector.reduce_sum(out=rd, in_=rs_d[:, 0:ndt], axis=mybir.AxisListType.X)
            rn = rsp.tile([P, 1], F32, tag="rn", name=f"rn{qi}")
            nc.vector.tensor_add(rn, ra, rd)
            ra = rn
        recb = rsp.tile([P, 1], BF16, tag="recb", name=f"recb{qi}")
        with nc.allow_low_precision(reason="bf16 softmax reciprocal, as before"):
            nc.vector.reciprocal(out=recb, in_=ra)
        return Es, recb

    def emit_colsum(qi, Es, recb):
        for m in range(8):
            wt, strip = w_ps[m // 4], 32 * (m % 4)
            nc.tensor.matmul(
                wt[strip : strip + 1, :],
                recb,
                Es[:, m * 512 : (m + 1) * 512],
                start=(qi == 0),
                stop=(qi == QTILES - 1),
                tile_position=(0, strip),
            )

    pending = {}
    for qi in range(QTILES):
        pending[qi] = emit_scores(qi)
        if qi - COLSUM_LAG in pending:
            emit_colsum(qi - COLSUM_LAG, *pending.pop(qi - COLSUM_LAG))
    for qi in sorted(pending):
        emit_colsum(qi, *pending.pop(qi))

    # ---- tail: w strips -> SBUF, PE-transpose to w^T, matvec against x,
    # then apply Wv to the pooled vector on-device.
    # w_sb_f[strip 32k, 512a + u] = w[t = 2048a + 512k + u]
    w_sb_f = const.tile([P, 1024], F32, name="w_sb_f")
    nc.vector.tensor_copy(out=w_sb_f[:, 0:512], in_=w_ps[0])
    nc.scalar.copy(out=w_sb_f[:, 512:1024], in_=w_ps[1])
    # transpose targets alternate between two psum tiles so each strided
    # cast (a read of tile c%2) doesn't serialize against the NEXT transpose
    # (a write to tile (c+1)%2) via tile-granular WAR tracking.
    wt_sb = const.tile([P, 32], BF16, name="wt_sb")
    tpa = pp.tile([P, 1536], F32, tag="ps", name="tp_a")
    tpb = pp.tile([P, 1536], F32, tag="ps", name="tp_b")
    for c in range(8):
        tp = (tpa, tpb)[c % 2]
        off = (c // 2) * P
        nc.tensor.transpose(
            out=tp[:, off : off + P],
            in_=w_sb_f[:, c * P : (c + 1) * P],
            identity=identity,
        )
        src = bass.AP(
            tensor=tp.tensor, offset=tp.offset + off, ap=[tp.ap[0], [32, 4]]
        )
        nc.vector.tensor_copy(out=wt_sb[:, c * 4 : (c + 1) * 4], in_=src)
    # pooled-x partials: 4 concurrent column strips at partitions {0,32,64,96}
    fin = wp.tile([P, 512], F32, tag="w0", name="fin")
    nc.vector.memset(fin[:, 0:E], 0.0)
    for col in range(32):
        c, k = divmod(col, 4)
        vidx = 16 * (c // 4) + 4 * k + (c % 4)
        strip = 32 * (col % 4)
        nc.tensor.matmul(
            fin[strip : strip + 1, 0:E],
            wt_sb[:, col : col + 1],
            xte[:, vidx, :],
            start=(col < 4),
            stop=(col >= 28),
            tile_position=(0, strip),
        )
    strips_sb = const.tile([P, E], F32, name="strips_sb")
    nc.vector.tensor_copy(out=strips_sb, in_=fin[:, 0:E])
    # transpose the 4 strip partials to [e-part, strip] and apply Wv
    ss4 = const.tile([P, 2, 4], BF16, name="ss4")
    for ei in range(2):
        tpe = pp.tile([P, 1536], F32, tag="ps", name=f"tpe{ei}")
        nc.tensor.transpose(
            out=tpe[:, 0:P], in_=strips_sb[:, ei * P : (ei + 1) * P], identity=identity
        )
        src = bass.AP(tensor=tpe.tensor, offset=tpe.offset, ap=[tpe.ap[0], [32, 4]])
        nc.vector.tensor_copy(out=ss4[:, ei, :], in_=src)
    pf = wp.tile([P, 512], F32, tag="w1", name="pf")
    for eo in range(2):
        for ei in range(2):
            nc.tensor.matmul(
                pf[:, eo * 4 : (eo + 1) * 4],
                wvb[:, ei, eo * P : (eo + 1) * P],
                ss4[:, ei, :],
                start=(ei == 0),
                stop=(ei == 1),
            )
    po = const.tile([P, 2], F32, name="po")
    for eo in range(2):
        nc.vector.reduce_sum(
            out=po[:, eo : eo + 1],
            in_=pf[:, eo * 4 : (eo + 1) * 4],
            axis=mybir.AxisListType.X,
        )
    nc.sync.dma_start(out=out_d[:, :], in_=po)


_NC_CACHE = None


def _build_nc():
    global _NC_CACHE
    if _NC_CACHE is None:
        from contextlib import ExitStack

        nc = bacc.Bacc("TRN2", target_bir_lowering=False, debug=False)
        with tile.TileContext(nc) as tc, ExitStack() as ctx:
            _emit(ctx, tc)
        nc.compile()
        _NC_CACHE = nc
    return _NC_CACHE


def _in_maps(inputs):
    import ml_dtypes

    bf16 = ml_dtypes.bfloat16
    f8 = ml_dtypes.float8_e4m3

    def to8(a):
        return np.clip(a, -240.0, 240.0).astype(f8)

    x = np.asarray(inputs["x"], dtype=np.float32)
    Wq = np.asarray(inputs["Wq"], dtype=np.float64)
    Wk = np.asarray(inputs["Wk"], dtype=np.float64)
    Wv = np.asarray(inputs["Wv"], dtype=np.float32)
    bq = np.asarray(inputs["bq"], dtype=np.float64)

    M16 = (16.0 * (Wq @ Wk.T)).astype(np.float32)
    u16 = (16.0 * (Wk @ bq)).astype(np.float32)
    m8 = np.ascontiguousarray(to8(M16).reshape(2, P, E).transpose(1, 0, 2))
    u16c = np.ascontiguousarray(u16.reshape(2, P).T)
    wvb = np.ascontiguousarray(Wv.astype(bf16).reshape(2, P, E).transpose(1, 0, 2))

    maps = []
    for c in range(N_CORES):
        b, h = divmod(c, 2)
        xr = np.roll(x[b], -h * HALF, axis=0)
        x8 = np.ascontiguousarray(
            to8(xr.T).reshape(2, P, S).transpose(1, 0, 2)
        )
        xte = np.ascontiguousarray(
            xr.astype(bf16).reshape(S // P, P, E).transpose(1, 0, 2)
        )
        maps.append({"x8": x8, "xte": xte, "m8": m8, "wvb": wvb, "u16c": u16c})
    return maps


def _combine(results, inputs):
    bv = np.asarray(inputs["bv"], dtype=np.float32).reshape(E)
    pooled = [np.asarray(r["out"], np.float32).T.reshape(E) for r in results]
    out = np.stack(
        [(pooled[2 * b] + pooled[2 * b + 1]) / S + bv for b in range(B)]
    )[:, None, :]
    return out.astype(np.float32)


def kernel(**inputs):
    from concourse.bass_utils import run_bass_kernel_spmd

    nc = _build_nc()
    res = run_bass_kernel_spmd(nc, _in_maps(inputs), core_ids=list(range(N_CORES)))
    return _combine(res.results, inputs)
